# revision 11
# baseline (speedup 1.0000x reference)
"""DepthDC fused kernel for 8 Trainium2 NeuronCores (bf16 pipeline).

Reference computation (N=2, C=64, H=W=256, d=2):
  patches[n,c,k,h,w] = xpad[n,c,h+ki*d, w+kj*d]   (k=3*ki+kj, pad d)
  out1 = sum_k patches * y.reshape(N,C,9,H,W)
  out  = leaky_relu(conv3x3(out1, fuse_w) + fuse_b, 0.2)

Sharding: 8 cores = batch(2) x H-quarters(4). Each core produces a
[64, 64, 256] output slab. Host slices overlapping (haloed, zero-padded)
input slabs per core, so no device collectives are needed.

Per-core layout: the 64 output rows split into two 32-row halves mapped
to SBUF partition halves (partition = c + 64*s). Everything ships in
bf16 (validated: absmax rel err ~5e-3 vs the 2e-2 gate), which halves
HBM traffic, doubles DVE throughput, and runs PE at 1 cycle/row:
  - host pre-layouts y as [128, 34, 9, 256] so each y chunk is ONE
    128-partition DMA with ~18KB contiguous per-partition descriptors
  - DVE: 3 multiplies (ki=0..2, innermost-contiguous, 2x bf16 mode)
         + 4 tree-adds reduce the 9 taps -> o1 (bf16)
  - PE:  3x3 dense conv as 9 accumulating bf16 matmuls (free dim 1024)
  - ACT: Lrelu(psum + bias, alpha=0.2) -> bf16 out tile
Output returns as bf16 [128, 32, 256] per core; host upcasts to fp32.
"""

import sys

sys.path.insert(0, "/opt/trn_rl_repo")

import numpy as np
import ml_dtypes

import concourse.bass as bass
import concourse.mybir as mybir
import concourse.tile as tile
from concourse import bacc
from concourse.bass_utils import run_bass_kernel_spmd

F32 = mybir.dt.float32
BF16 = mybir.dt.bfloat16
AF = mybir.ActivationFunctionType
ALU = mybir.AluOpType
BDT = ml_dtypes.bfloat16

N, C, H, W = 2, 64, 256, 256
D = 2  # dilation == pad
NCORES = 8
HB = 64          # output rows per core
HH = 32          # output rows per half
Q = HH + 2       # out1 rows per half (34)
XR = Q + 4       # x rows per half block (38)
XW = W + 2 * D   # padded x width (260)
OW = W + 2       # padded out1 width (258)
RC = 8           # out1 rows per reduce chunk
RCHUNKS = (8, 8, 8, 8, 2)   # sum = 34
CC = 8           # out rows per conv chunk (4 chunks of 8)


def _build_program():
    nc = bacc.Bacc("TRN2", target_bir_lowering=False, debug=False,
                   num_devices=NCORES)

    xb_d = nc.dram_tensor("xb", [128, XR, XW], BF16, kind="ExternalInput").ap()
    yb_d = nc.dram_tensor("yb", [128, Q, 9, W], BF16,
                          kind="ExternalInput").ap()
    wt_d = nc.dram_tensor("wt", [9, 128, 128], BF16, kind="ExternalInput").ap()
    b_d = nc.dram_tensor("bias", [128, 1], F32, kind="ExternalInput").ap()
    ob_d = nc.dram_tensor("out", [128, HH, W], BF16,
                          kind="ExternalOutput").ap()

    with tile.TileContext(nc) as tc:
        from contextlib import ExitStack
        with ExitStack() as ctx:
            const = ctx.enter_context(tc.tile_pool(name="const", bufs=1))
            y_pool = ctx.enter_context(tc.tile_pool(name="y_pool", bufs=2))
            p_pool = ctx.enter_context(tc.tile_pool(name="p_pool", bufs=1))
            s_pool = ctx.enter_context(tc.tile_pool(name="s_pool", bufs=1))
            o_pool = ctx.enter_context(tc.tile_pool(name="o_pool", bufs=2))
            v_pool = ctx.enter_context(tc.tile_pool(name="v_pool", bufs=2))
            ps_pool = ctx.enter_context(
                tc.tile_pool(name="ps_pool", bufs=2, space="PSUM"))

            w_sb = const.tile([128, 9, 128], BF16, name="w_sb")
            nc.sync.dma_start(w_sb[:], wt_d.rearrange("t p m -> p t m"))
            b_sb = const.tile([128, 1], F32, name="b_sb")
            nc.sync.dma_start(b_sb[:], b_d)
            x_sb = const.tile([128, XR, XW], BF16, name="x_sb")
            nc.sync.dma_start(x_sb[:], xb_d)
            o1_sb = const.tile([128, Q, OW], BF16, name="o1_sb")
            # zero the conv W-padding columns once
            nc.vector.memset(o1_sb[:, :, 0:1], 0.0)
            nc.vector.memset(o1_sb[:, :, OW - 1:OW], 0.0)
            # Wait-merge scratch: one cheap DVE copy per const DMA converts
            # DMA-completion semaphores into DVE program order, so compute
            # instructions never need more than 1 foreign wait sem.
            scr = const.tile([128, 8], F32, name="scr")
            nc.vector.tensor_copy(scr[:, 0:1], x_sb[:, 0, 0:2].bitcast(F32))
            nc.vector.tensor_copy(scr[:, 1:2],
                                  x_sb[:, XR - 1, XW - 2:XW].bitcast(F32))
            nc.vector.tensor_copy(scr[:, 2:3], w_sb[:, 0, 0:2].bitcast(F32))
            nc.vector.tensor_copy(scr[:, 3:4], b_sb[:, 0:1])

            def reduce_chunk(c):
                q0 = sum(RCHUNKS[:c])
                rc = RCHUNKS[c]
                y_t = y_pool.tile([128, RC, 9, W], BF16, name="y_t", tag="y_t")
                nc.sync.dma_start(y_t[:, 0:rc], yb_d[:, q0:q0 + rc])
                p_t = p_pool.tile([128, RC, 9, W], BF16, name="p_t", tag="p_t")
                # prod[p, r, 3ki+kj, w] = x[p, q0+r+2ki, 2kj+w] * y_t[...]
                for ki in range(3):
                    v = x_sb[:, q0 + 2 * ki: q0 + 2 * ki + rc, 0:W]
                    a = [list(p) for p in v.ap]
                    x_view = bass.AP(
                        v.tensor, v.offset, [a[0], a[1], [2, 3], a[2]])
                    nc.vector.tensor_tensor(
                        p_t[:, 0:rc, 3 * ki:3 * ki + 3, :], x_view,
                        y_t[:, 0:rc, 3 * ki:3 * ki + 3, :], ALU.mult)
                # tree-add the 9 tap planes -> o1 (bf16); the big first
                # stage runs on the otherwise-idle GpSimd engine
                s_t = s_pool.tile([128, RC, 4, W], BF16, name="s_t",
                                  tag="s_t")
                nc.gpsimd.tensor_tensor(
                    s_t[:, 0:rc], p_t[:, 0:rc, 0:4, :],
                    p_t[:, 0:rc, 4:8, :], ALU.add)
                nc.vector.tensor_tensor(
                    s_t[:, 0:rc, 0:2, :], s_t[:, 0:rc, 0:2, :],
                    s_t[:, 0:rc, 2:4, :], ALU.add)
                nc.vector.tensor_tensor(
                    s_t[:, 0:rc, 0:1, :], s_t[:, 0:rc, 0:1, :],
                    s_t[:, 0:rc, 1:2, :], ALU.add)
                nc.vector.tensor_tensor(
                    o1_sb[:, q0:q0 + rc, 1:W + 1], s_t[:, 0:rc, 0, :],
                    p_t[:, 0:rc, 8, :], ALU.add)

            def conv_chunk(j):
                m0 = CC * j
                ps = ps_pool.tile([128, CC, W], F32, name="ps", tag="ps")
                for t in range(9):
                    i3, j3 = divmod(t, 3)
                    for h in (0, 2, 4, 6):
                        nc.tensor.matmul(
                            ps[:, h:h + 2, :], lhsT=w_sb[:, t],
                            rhs=o1_sb[:, m0 + i3 + h: m0 + i3 + h + 2,
                                      j3: j3 + W],
                            start=(t == 0), stop=(t == 8))
                v_t = v_pool.tile([128, CC, W], F32, name="v_t", tag="v_t")
                nc.scalar.add(v_t[:], ps[:], b_sb[:, 0:1])
                o_t = o_pool.tile([128, CC, W], BF16, name="o_t", tag="o_t")
                # leaky_relu(v) = max(v, 0.2*v)
                nc.vector.scalar_tensor_tensor(
                    o_t[:], v_t[:], 0.2, v_t[:], ALU.mult, ALU.max)
                nc.sync.dma_start(ob_d[:, m0:m0 + CC, :], o_t[:])

            # pipeline: conv chunk j needs o1 rows m0..m0+9, i.e. reduce
            # chunks j and j+1
            reduce_chunk(0)
            for c in range(1, len(RCHUNKS)):
                reduce_chunk(c)
                conv_chunk(c - 1)

    nc.compile()
    return nc


_PROGRAM = None


def _get_program():
    global _PROGRAM
    if _PROGRAM is None:
        _PROGRAM = _build_program()
    return _PROGRAM


def make_in_maps(x, y, fuse_w, fuse_b):
    x = np.asarray(x, dtype=np.float32)
    y = np.asarray(y, dtype=np.float32)
    fuse_w = np.asarray(fuse_w, dtype=np.float32)
    fuse_b = np.asarray(fuse_b, dtype=np.float32)

    # block-diagonal conv weights: each partition half (h-half of the
    # slab) contracts with its own copy of W_tap in one K=128 matmul
    wt = np.zeros((9, 128, 128), BDT)
    for t in range(9):
        i, j = divmod(t, 3)
        wtap = fuse_w[:, :, i, j].T.astype(BDT)  # [c_in, c_out]
        wt[t, 0:64, 0:64] = wtap
        wt[t, 64:128, 64:128] = wtap
    bias = np.concatenate([fuse_b, fuse_b]).astype(np.float32)[:, None]

    x16 = x.astype(BDT)
    # y in [n, c, h, k, w] order so per-partition chunks are contiguous
    yt = np.ascontiguousarray(
        y.astype(BDT).reshape(N, C, 9, H, W).transpose(0, 1, 3, 2, 4))

    in_maps = []
    for core in range(NCORES):
        n, hb = divmod(core, 4)
        h0 = hb * HB
        xb = np.zeros((128, XR, XW), BDT)
        yb = np.zeros((128, Q, 9, W), BDT)
        for s in (0, 1):
            base = h0 + HH * s
            # x rows base-3 .. base+34 (XR=38)
            r0, r1 = base - 3, base + XR - 3
            s0, s1 = max(r0, 0), min(r1, H)
            xb[64 * s:64 * s + 64, s0 - r0:s1 - r0, D:D + W] = x16[n, :, s0:s1]
            # y rows base-1 .. base+32 (Q=34)
            r0y, r1y = base - 1, base + Q - 1
            s0y, s1y = max(r0y, 0), min(r1y, H)
            yb[64 * s:64 * s + 64, s0y - r0y:s1y - r0y] = \
                yt[n, :, s0y:s1y]
        in_maps.append({"xb": xb, "yb": yb, "wt": wt, "bias": bias})
    return in_maps


def run(x, y, fuse_w, fuse_b, trace=False, **kw):
    nc = _get_program()
    in_maps = make_in_maps(x, y, fuse_w, fuse_b)
    res = run_bass_kernel_spmd(nc, in_maps, list(range(NCORES)),
                               trace=trace, **kw)
    out = np.empty((N, C, H, W), np.float32)
    for core in range(NCORES):
        n, hb = divmod(core, 4)
        ob = np.asarray(res.results[core]["out"]).astype(np.float32)
        out[n, :, hb * HB:hb * HB + HH, :] = ob[0:64]
        out[n, :, hb * HB + HH:hb * HB + HB, :] = ob[64:128]
    return out, res


def kernel(x, y, fuse_w, fuse_b):
    out, _ = run(x, y, fuse_w, fuse_b, trace=False)
    return out


# revision 12
# speedup vs baseline: 1.8222x; 1.8222x over previous
"""DepthDC fused kernel for 8 Trainium2 NeuronCores (bf16 pipeline).

Reference computation (N=2, C=64, H=W=256, d=2):
  patches[n,c,k,h,w] = xpad[n,c,h+ki*d, w+kj*d]   (k=3*ki+kj, pad d)
  out1 = sum_k patches * y.reshape(N,C,9,H,W)
  out  = leaky_relu(conv3x3(out1, fuse_w) + fuse_b, 0.2)

Sharding: 8 cores = batch(2) x H-quarters(4). Each core produces a
[64, 64, 256] output slab. Host slices overlapping (haloed, zero-padded)
input slabs per core, so no device collectives are needed.

Per-core layout: the 64 output rows split into two 32-row halves mapped
to SBUF partition halves (partition = c + 64*s). Everything ships in
bf16 (validated: absmax rel err ~5e-3 vs the 2e-2 gate), which halves
HBM traffic and runs the PE at 1 cycle/row:
  - host pre-layouts y as [128, 34, 9, 256] so each y chunk is ONE
    128-partition DMA with contiguous per-partition descriptors
  - DVE: 3 multiplies per chunk (ki=0..2, innermost-contiguous)
  - PE:  9-tap k-reduction via accumulating identity matmuls (PSUM),
         then the 3x3 dense conv as 9 accumulating bf16 matmuls
  - ACT: PSUM->o1 (bf16 cast) copies and the conv bias add
  - DVE: leaky_relu(v) = max(v, 0.2*v) epilogue
Output returns as bf16 [128, 32, 256] per core; host upcasts to fp32.
"""

import sys

sys.path.insert(0, "/opt/trn_rl_repo")

import numpy as np
import ml_dtypes

import concourse.bass as bass
import concourse.mybir as mybir
import concourse.tile as tile
from concourse import bacc
from concourse.bass_utils import run_bass_kernel_spmd

F32 = mybir.dt.float32
BF16 = mybir.dt.bfloat16
AF = mybir.ActivationFunctionType
ALU = mybir.AluOpType
BDT = ml_dtypes.bfloat16

N, C, H, W = 2, 64, 256, 256
D = 2  # dilation == pad
NCORES = 8
HB = 64          # output rows per core
HH = 32          # output rows per half
Q = HH + 2       # out1 rows per half (34)
XR = Q + 4       # x rows per half block (38)
XW = W + 2 * D   # padded x width (260)
OW = W + 2       # padded out1 width (258)
RC = 4           # out1 rows per reduce chunk (9 chunks: 8x4 + 1x2)
NRED = 9
CC = 4           # out rows per conv chunk (8 chunks)
NCONV = 8


def _build_program():
    nc = bacc.Bacc("TRN2", target_bir_lowering=False, debug=False,
                   num_devices=NCORES)

    xb_d = nc.dram_tensor("xb", [128, XR, XW], BF16, kind="ExternalInput").ap()
    yb_d = nc.dram_tensor("yb", [128, Q, 9, W], BF16,
                          kind="ExternalInput").ap()
    wt_d = nc.dram_tensor("wt", [9, 128, 128], BF16, kind="ExternalInput").ap()
    id_d = nc.dram_tensor("ident", [128, 128], BF16,
                          kind="ExternalInput").ap()
    b_d = nc.dram_tensor("bias", [128, 1], F32, kind="ExternalInput").ap()
    ob_d = nc.dram_tensor("out", [128, HH, W], BF16,
                          kind="ExternalOutput").ap()

    with tile.TileContext(nc) as tc:
        from contextlib import ExitStack
        with ExitStack() as ctx:
            const = ctx.enter_context(tc.tile_pool(name="const", bufs=1))
            y_pool = ctx.enter_context(tc.tile_pool(name="y_pool", bufs=3))
            p_pool = ctx.enter_context(tc.tile_pool(name="p_pool", bufs=2))
            o_pool = ctx.enter_context(tc.tile_pool(name="o_pool", bufs=2))
            v_pool = ctx.enter_context(tc.tile_pool(name="v_pool", bufs=2))
            psr_pool = ctx.enter_context(
                tc.tile_pool(name="psr_pool", bufs=2, space="PSUM"))
            ps_pool = ctx.enter_context(
                tc.tile_pool(name="ps_pool", bufs=2, space="PSUM"))

            w_sb = const.tile([128, 9, 128], BF16, name="w_sb")
            nc.sync.dma_start(w_sb[:], wt_d.rearrange("t p m -> p t m"))
            id_sb = const.tile([128, 128], BF16, name="id_sb")
            nc.sync.dma_start(id_sb[:], id_d)
            b_sb = const.tile([128, 1], F32, name="b_sb")
            nc.sync.dma_start(b_sb[:], b_d)
            x_sb = const.tile([128, XR, XW], BF16, name="x_sb")
            nc.sync.dma_start(x_sb[:], xb_d)
            o1_sb = const.tile([128, Q, OW], BF16, name="o1_sb")
            # zero the conv W-padding columns once
            nc.vector.memset(o1_sb[:, :, 0:1], 0.0)
            nc.vector.memset(o1_sb[:, :, OW - 1:OW], 0.0)
            # Wait-merge scratch: one cheap DVE copy per const DMA converts
            # DMA-completion semaphores into DVE program order, so compute
            # instructions never need more than 1 foreign wait sem.
            scr = const.tile([128, 8], F32, name="scr")
            nc.vector.tensor_copy(scr[:, 0:1], x_sb[:, 0, 0:2].bitcast(F32))
            nc.vector.tensor_copy(scr[:, 1:2],
                                  x_sb[:, XR - 1, XW - 2:XW].bitcast(F32))
            nc.vector.tensor_copy(scr[:, 2:3], w_sb[:, 0, 0:2].bitcast(F32))
            nc.vector.tensor_copy(scr[:, 3:4], id_sb[:, 0:2].bitcast(F32))
            nc.vector.tensor_copy(scr[:, 4:5], b_sb[:, 0:1])

            def reduce_chunk(c):
                q0 = RC * c
                rc = min(RC, Q - q0)
                y_t = y_pool.tile([128, RC, 9, W], BF16, name="y_t", tag="y_t")
                nc.sync.dma_start(y_t[:, 0:rc], yb_d[:, q0:q0 + rc])
                p_t = p_pool.tile([128, RC, 9, W], BF16, name="p_t", tag="p_t")
                # prod[p, r, 3ki+kj, w] = x[p, q0+r+2ki, 2kj+w] * y_t[...]
                for ki in range(3):
                    v = x_sb[:, q0 + 2 * ki: q0 + 2 * ki + rc, 0:W]
                    a = [list(p) for p in v.ap]
                    x_view = bass.AP(
                        v.tensor, v.offset, [a[0], a[1], [2, 3], a[2]])
                    nc.vector.tensor_tensor(
                        p_t[:, 0:rc, 3 * ki:3 * ki + 3, :], x_view,
                        y_t[:, 0:rc, 3 * ki:3 * ki + 3, :], ALU.mult)
                # k-reduction: 9 accumulating identity matmuls into PSUM
                psr = psr_pool.tile([128, RC, W], F32, name="psr", tag="psr")
                for k in range(9):
                    for h in range(0, rc, 2):
                        nc.tensor.matmul(
                            psr[:, h:h + 2, :], lhsT=id_sb[:],
                            rhs=p_t[:, h:h + 2, k, :],
                            start=(k == 0), stop=(k == 8))
                nc.scalar.copy(o1_sb[:, q0:q0 + rc, 1:W + 1], psr[:, 0:rc, :])

            def conv_chunk(j):
                m0 = CC * j
                ps = ps_pool.tile([128, CC, W], F32, name="ps", tag="ps")
                for t in range(9):
                    i3, j3 = divmod(t, 3)
                    for h in (0, 2):
                        nc.tensor.matmul(
                            ps[:, h:h + 2, :], lhsT=w_sb[:, t],
                            rhs=o1_sb[:, m0 + i3 + h: m0 + i3 + h + 2,
                                      j3: j3 + W],
                            start=(t == 0), stop=(t == 8))
                v_t = v_pool.tile([128, CC, W], BF16, name="v_t", tag="v_t")
                nc.scalar.add(v_t[:], ps[:], b_sb[:, 0:1])
                o_t = o_pool.tile([128, CC, W], BF16, name="o_t", tag="o_t")
                # leaky_relu(v) = max(v, 0.2*v)
                nc.vector.scalar_tensor_tensor(
                    o_t[:], v_t[:], 0.2, v_t[:], ALU.mult, ALU.max)
                nc.sync.dma_start(ob_d[:, m0:m0 + CC, :], o_t[:])

            # pipeline: conv chunk j needs o1 rows 4j..4j+5, i.e. reduce
            # chunks j and j+1; keep one extra chunk of lookahead
            reduce_chunk(0)
            reduce_chunk(1)
            for j in range(NCONV):
                if j + 2 < NRED:
                    reduce_chunk(j + 2)
                conv_chunk(j)

    nc.compile()
    return nc


_PROGRAM = None


def _get_program():
    global _PROGRAM
    if _PROGRAM is None:
        _PROGRAM = _build_program()
    return _PROGRAM


def make_in_maps(x, y, fuse_w, fuse_b):
    x = np.asarray(x, dtype=np.float32)
    y = np.asarray(y, dtype=np.float32)
    fuse_w = np.asarray(fuse_w, dtype=np.float32)
    fuse_b = np.asarray(fuse_b, dtype=np.float32)

    # block-diagonal conv weights: each partition half (h-half of the
    # slab) contracts with its own copy of W_tap in one K=128 matmul
    wt = np.zeros((9, 128, 128), BDT)
    for t in range(9):
        i, j = divmod(t, 3)
        wtap = fuse_w[:, :, i, j].T.astype(BDT)  # [c_in, c_out]
        wt[t, 0:64, 0:64] = wtap
        wt[t, 64:128, 64:128] = wtap
    ident = np.eye(128, dtype=BDT)
    bias = np.concatenate([fuse_b, fuse_b]).astype(np.float32)[:, None]

    x16 = x.astype(BDT)
    # y in [n, c, h, k, w] order so per-partition chunks are contiguous
    yt = np.ascontiguousarray(
        y.astype(BDT).reshape(N, C, 9, H, W).transpose(0, 1, 3, 2, 4))

    in_maps = []
    for core in range(NCORES):
        n, hb = divmod(core, 4)
        h0 = hb * HB
        xb = np.zeros((128, XR, XW), BDT)
        yb = np.zeros((128, Q, 9, W), BDT)
        for s in (0, 1):
            base = h0 + HH * s
            # x rows base-3 .. base+34 (XR=38)
            r0, r1 = base - 3, base + XR - 3
            s0, s1 = max(r0, 0), min(r1, H)
            xb[64 * s:64 * s + 64, s0 - r0:s1 - r0, D:D + W] = x16[n, :, s0:s1]
            # y rows base-1 .. base+32 (Q=34)
            r0y, r1y = base - 1, base + Q - 1
            s0y, s1y = max(r0y, 0), min(r1y, H)
            yb[64 * s:64 * s + 64, s0y - r0y:s1y - r0y] = \
                yt[n, :, s0y:s1y]
        in_maps.append({"xb": xb, "yb": yb, "wt": wt, "ident": ident,
                        "bias": bias})
    return in_maps


def run(x, y, fuse_w, fuse_b, trace=False, **kw):
    nc = _get_program()
    in_maps = make_in_maps(x, y, fuse_w, fuse_b)
    res = run_bass_kernel_spmd(nc, in_maps, list(range(NCORES)),
                               trace=trace, **kw)
    out = np.empty((N, C, H, W), np.float32)
    for core in range(NCORES):
        n, hb = divmod(core, 4)
        ob = np.asarray(res.results[core]["out"]).astype(np.float32)
        out[n, :, hb * HB:hb * HB + HH, :] = ob[0:64]
        out[n, :, hb * HB + HH:hb * HB + HB, :] = ob[64:128]
    return out, res


def kernel(x, y, fuse_w, fuse_b):
    out, _ = run(x, y, fuse_w, fuse_b, trace=False)
    return out


# revision 16
# speedup vs baseline: 1.8617x; 1.0217x over previous
"""DepthDC fused kernel for 8 Trainium2 NeuronCores (bf16 pipeline).

Reference computation (N=2, C=64, H=W=256, d=2):
  patches[n,c,k,h,w] = xpad[n,c,h+ki*d, w+kj*d]   (k=3*ki+kj, pad d)
  out1 = sum_k patches * y.reshape(N,C,9,H,W)
  out  = leaky_relu(conv3x3(out1, fuse_w) + fuse_b, 0.2)

Sharding: 8 cores = batch(2) x H-quarters(4). Each core produces a
[64, 64, 256] output slab. Host slices overlapping (haloed, zero-padded)
input slabs per core, so no device collectives are needed.

Per-core layout: the 64 output rows split into two 32-row halves mapped
to SBUF partition halves (partition = c + 64*s). Everything ships in
bf16 (validated: absmax rel err ~5e-3 vs the 2e-2 gate), which halves
HBM traffic and runs the PE at 1 cycle/row:
  - host pre-layouts y as [128, 34, 9, 256] so each y chunk is ONE
    128-partition DMA with contiguous per-partition descriptors
  - DVE: 3 multiplies per chunk (ki=0..2, innermost-contiguous)
  - PE:  9-tap k-reduction via accumulating identity matmuls (PSUM),
         then the 3x3 dense conv as 9 accumulating bf16 matmuls
  - ACT: PSUM->o1 (bf16 cast) copies and the conv bias add
  - DVE: leaky_relu(v) = max(v, 0.2*v) epilogue
Output returns as bf16 [128, 32, 256] per core; host upcasts to fp32.
"""

import sys

sys.path.insert(0, "/opt/trn_rl_repo")

import numpy as np
import ml_dtypes

import concourse.bass as bass
import concourse.mybir as mybir
import concourse.tile as tile
from concourse import bacc
from concourse.bass_utils import run_bass_kernel_spmd

F32 = mybir.dt.float32
BF16 = mybir.dt.bfloat16
AF = mybir.ActivationFunctionType
ALU = mybir.AluOpType
BDT = ml_dtypes.bfloat16

N, C, H, W = 2, 64, 256, 256
D = 2  # dilation == pad
NCORES = 8
HB = 64          # output rows per core
HH = 32          # output rows per half
Q = HH + 2       # out1 rows per half (34)
XR = Q + 4       # x rows per half block (38)
XW = W + 2 * D   # padded x width (260)
OW = W + 2       # padded out1 width (258)
RC = 4           # out1 rows per reduce chunk (9 chunks: 8x4 + 1x2)
NRED = 9
CC = 4           # out rows per conv chunk (8 chunks)
NCONV = 8


def _build_program():
    nc = bacc.Bacc("TRN2", target_bir_lowering=False, debug=False,
                   num_devices=NCORES)

    xb_d = nc.dram_tensor("xb", [128, XR, XW], BF16, kind="ExternalInput").ap()
    yb_d = nc.dram_tensor("yb", [128, Q, 9, W], BF16,
                          kind="ExternalInput").ap()
    wt_d = nc.dram_tensor("wt", [9, 128, 128], BF16, kind="ExternalInput").ap()
    id_d = nc.dram_tensor("ident", [128, 128], BF16,
                          kind="ExternalInput").ap()
    b_d = nc.dram_tensor("bias", [128, 1], F32, kind="ExternalInput").ap()
    ob_d = nc.dram_tensor("out", [128, HH, W], BF16,
                          kind="ExternalOutput").ap()

    with tile.TileContext(nc) as tc:
        from contextlib import ExitStack
        with ExitStack() as ctx:
            const = ctx.enter_context(tc.tile_pool(name="const", bufs=1))
            y_pool = ctx.enter_context(tc.tile_pool(name="y_pool", bufs=3))
            p_pool = ctx.enter_context(tc.tile_pool(name="p_pool", bufs=2))
            o_pool = ctx.enter_context(tc.tile_pool(name="o_pool", bufs=2))
            v_pool = ctx.enter_context(tc.tile_pool(name="v_pool", bufs=2))
            psr_pool = ctx.enter_context(
                tc.tile_pool(name="psr_pool", bufs=2, space="PSUM"))
            ps_pool = ctx.enter_context(
                tc.tile_pool(name="ps_pool", bufs=2, space="PSUM"))

            w_sb = const.tile([128, 9, 128], BF16, name="w_sb")
            nc.sync.dma_start(w_sb[:], wt_d.rearrange("t p m -> p t m"))
            id_sb = const.tile([128, 128], BF16, name="id_sb")
            nc.sync.dma_start(id_sb[:], id_d)
            b_sb = const.tile([128, 1], F32, name="b_sb")
            nc.sync.dma_start(b_sb[:], b_d)
            b2_sb = const.tile([128, 1], F32, name="b2_sb")
            nc.vector.tensor_scalar_mul(b2_sb[:], b_sb[:], 0.2)
            x_sb = const.tile([128, XR, XW], BF16, name="x_sb")
            # head rows cover reduce chunks 0-1; tail DMA is issued after
            # y1 so the pipeline starts sooner
            XH = 12
            nc.sync.dma_start(x_sb[:, 0:XH], xb_d[:, 0:XH])
            o1_sb = const.tile([128, Q, OW], BF16, name="o1_sb")
            # zero the conv W-padding columns once
            nc.vector.memset(o1_sb[:, :, 0:1], 0.0)
            nc.vector.memset(o1_sb[:, :, OW - 1:OW], 0.0)
            # Wait-merge scratch: one cheap DVE copy per const DMA converts
            # DMA-completion semaphores into DVE program order, so compute
            # instructions never need more than 1 foreign wait sem.
            scr = const.tile([128, 8], F32, name="scr")
            nc.vector.tensor_copy(scr[:, 0:1], x_sb[:, 0, 0:2].bitcast(F32))
            nc.vector.tensor_copy(scr[:, 2:3], w_sb[:, 0, 0:2].bitcast(F32))
            nc.vector.tensor_copy(scr[:, 3:4], id_sb[:, 0:2].bitcast(F32))
            nc.vector.tensor_copy(scr[:, 4:5], b_sb[:, 0:1])

            def reduce_chunk(c):
                q0 = RC * c
                rc = min(RC, Q - q0)
                if c == 2:
                    # x tail lands behind y0/y1 on the DMA queue
                    nc.sync.dma_start(x_sb[:, 12:XR], xb_d[:, 12:XR])
                    nc.vector.tensor_copy(
                        scr[:, 1:2], x_sb[:, XR - 1, XW - 2:XW].bitcast(F32))
                y_t = y_pool.tile([128, RC, 9, W], BF16, name="y_t", tag="y_t")
                nc.sync.dma_start(y_t[:, 0:rc], yb_d[:, q0:q0 + rc])
                p_t = p_pool.tile([128, RC, 9, W], BF16, name="p_t", tag="p_t")
                # prod[p, r, 3ki+kj, w] = x[p, q0+r+2ki, 2kj+w] * y_t[...]
                for ki in range(3):
                    v = x_sb[:, q0 + 2 * ki: q0 + 2 * ki + rc, 0:W]
                    a = [list(p) for p in v.ap]
                    x_view = bass.AP(
                        v.tensor, v.offset, [a[0], a[1], [2, 3], a[2]])
                    nc.vector.tensor_tensor(
                        p_t[:, 0:rc, 3 * ki:3 * ki + 3, :], x_view,
                        y_t[:, 0:rc, 3 * ki:3 * ki + 3, :], ALU.mult)
                # pre-add planes (5,6) and (7,8) on DVE to offload the PE
                nc.vector.tensor_tensor(
                    p_t[:, 0:rc, 5, :], p_t[:, 0:rc, 5, :],
                    p_t[:, 0:rc, 6, :], ALU.add)
                nc.vector.tensor_tensor(
                    p_t[:, 0:rc, 7, :], p_t[:, 0:rc, 7, :],
                    p_t[:, 0:rc, 8, :], ALU.add)
                # k-reduction: 7 accumulating identity matmuls into PSUM
                psr = psr_pool.tile([128, RC, W], F32, name="psr", tag="psr")
                planes = (0, 1, 2, 3, 4, 5, 7)
                for k in planes:
                    for h in range(0, rc, 2):
                        nc.tensor.matmul(
                            psr[:, h:h + 2, :], lhsT=id_sb[:],
                            rhs=p_t[:, h:h + 2, k, :],
                            start=(k == 0), stop=(k == 7))
                nc.scalar.copy(o1_sb[:, q0:q0 + rc, 1:W + 1], psr[:, 0:rc, :])

            def conv_chunk(j):
                m0 = CC * j
                ps = ps_pool.tile([128, CC, W], F32, name="ps", tag="ps")
                for t in range(9):
                    i3, j3 = divmod(t, 3)
                    for h in (0, 2):
                        nc.tensor.matmul(
                            ps[:, h:h + 2, :], lhsT=w_sb[:, t],
                            rhs=o1_sb[:, m0 + i3 + h: m0 + i3 + h + 2,
                                      j3: j3 + W],
                            start=(t == 0), stop=(t == 8))
                # leaky_relu(ps + b) = max(ps + b, 0.2*ps + 0.2*b):
                # both linear forms on ACT, single bf16 max on DVE
                v_t = v_pool.tile([128, 2, CC, W], BF16, name="v_t",
                                  tag="v_t")
                nc.scalar.add(v_t[:, 0], ps[:], b_sb[:, 0:1])
                nc.scalar.activation(v_t[:, 1], ps[:], AF.Identity,
                                     bias=b2_sb[:, 0:1], scale=0.2)
                o_t = o_pool.tile([128, CC, W], BF16, name="o_t", tag="o_t")
                nc.vector.tensor_tensor(o_t[:], v_t[:, 0], v_t[:, 1],
                                        ALU.max)
                nc.sync.dma_start(ob_d[:, m0:m0 + CC, :], o_t[:])

            # pipeline: conv chunk j needs o1 rows 4j..4j+5, i.e. reduce
            # chunks j and j+1; keep one extra chunk of lookahead
            reduce_chunk(0)
            reduce_chunk(1)
            for j in range(NCONV):
                if j + 2 < NRED:
                    reduce_chunk(j + 2)
                conv_chunk(j)

    nc.compile()
    return nc


_PROGRAM = None


def _get_program():
    global _PROGRAM
    if _PROGRAM is None:
        _PROGRAM = _build_program()
    return _PROGRAM


def make_in_maps(x, y, fuse_w, fuse_b):
    x = np.asarray(x, dtype=np.float32)
    y = np.asarray(y, dtype=np.float32)
    fuse_w = np.asarray(fuse_w, dtype=np.float32)
    fuse_b = np.asarray(fuse_b, dtype=np.float32)

    # block-diagonal conv weights: each partition half (h-half of the
    # slab) contracts with its own copy of W_tap in one K=128 matmul
    wt = np.zeros((9, 128, 128), BDT)
    for t in range(9):
        i, j = divmod(t, 3)
        wtap = fuse_w[:, :, i, j].T.astype(BDT)  # [c_in, c_out]
        wt[t, 0:64, 0:64] = wtap
        wt[t, 64:128, 64:128] = wtap
    ident = np.eye(128, dtype=BDT)
    bias = np.concatenate([fuse_b, fuse_b]).astype(np.float32)[:, None]

    x16 = x.astype(BDT)
    # y in [n, c, h, k, w] order so per-partition chunks are contiguous
    yt = np.ascontiguousarray(
        y.astype(BDT).reshape(N, C, 9, H, W).transpose(0, 1, 3, 2, 4))

    in_maps = []
    for core in range(NCORES):
        n, hb = divmod(core, 4)
        h0 = hb * HB
        xb = np.zeros((128, XR, XW), BDT)
        yb = np.zeros((128, Q, 9, W), BDT)
        for s in (0, 1):
            base = h0 + HH * s
            # x rows base-3 .. base+34 (XR=38)
            r0, r1 = base - 3, base + XR - 3
            s0, s1 = max(r0, 0), min(r1, H)
            xb[64 * s:64 * s + 64, s0 - r0:s1 - r0, D:D + W] = x16[n, :, s0:s1]
            # y rows base-1 .. base+32 (Q=34)
            r0y, r1y = base - 1, base + Q - 1
            s0y, s1y = max(r0y, 0), min(r1y, H)
            yb[64 * s:64 * s + 64, s0y - r0y:s1y - r0y] = \
                yt[n, :, s0y:s1y]
        in_maps.append({"xb": xb, "yb": yb, "wt": wt, "ident": ident,
                        "bias": bias})
    return in_maps


def run(x, y, fuse_w, fuse_b, trace=False, **kw):
    nc = _get_program()
    in_maps = make_in_maps(x, y, fuse_w, fuse_b)
    res = run_bass_kernel_spmd(nc, in_maps, list(range(NCORES)),
                               trace=trace, **kw)
    out = np.empty((N, C, H, W), np.float32)
    for core in range(NCORES):
        n, hb = divmod(core, 4)
        ob = np.asarray(res.results[core]["out"]).astype(np.float32)
        out[n, :, hb * HB:hb * HB + HH, :] = ob[0:64]
        out[n, :, hb * HB + HH:hb * HB + HB, :] = ob[64:128]
    return out, res


def kernel(x, y, fuse_w, fuse_b):
    out, _ = run(x, y, fuse_w, fuse_b, trace=False)
    return out
